# revision 15
# baseline (speedup 1.0000x reference)
"""Trainium2 Bass kernel for FFN (additive) attention.

Reference computation (B=16, S=8192, D=512, H=512):
    q_h = query @ Wq.T + bq                      # (B, H)
    k_h = einsum("bsd,hd->bsh", key, Wk) + bk    # (B, S, H)
    attn_weight = einsum("bsh,h->bs", q_h[:,None,:] + k_h, w_out)
    attn_weight = where(mask, -1e18, attn_weight)
    attn_prob = softmax(attn_weight, -1)
    attn = einsum("bsh,bs->bh", k_h, attn_prob)
    return attn, attn_weight

Key identity: with v = Wk.T @ w_out (D,) and qs[b] = w_out . (Wq@q[b] + bq + bk),
    attn_weight[b,s] = v . key[b,s] + qs[b]
and softmax is invariant to the per-row constant qs[b], while
    attn[b] = Wk @ (sum_s p[b,s] key[b,s]) + bk.
So the device streams `key` (the 256MB input) exactly once:
    pass 1 (DVE): raw[b,s] = v . key[b,s] via one fused
                  scalar_tensor_tensor(+accum) per [128,512] tile on the
                  natural [s-partition, d-free] layout
    e = exp(raw + mask*(-1e18)) on ACT, mask folded into the per-tile
        per-partition activation bias  (no max subtraction: |raw| <~ 45)
    pass 2 (PE):  ctx[b,d] = sum_s e[b,s] key[b,s,d] as fp32r matmuls with
                  e as the 1-column stationary operand
Host applies the mask to the raw scores and finishes with the tiny (D,H)
projections in float64.

DMA queues: key loads own the Sync FIFO (so prefetch never blocks behind
output DMAs); outputs issue from the ACT HWDGE queue.

Sharding: data-parallel over batch, 2 batches per core on 8 cores.
Device layouts use s = 128*t + p (p = partition, t = 0..63 column).
"""

from contextlib import ExitStack

import numpy as np

import concourse.bass as bass
import concourse.bacc as bacc
import concourse.tile as tile
import concourse.mybir as mybir
from concourse.bass_utils import run_bass_kernel_spmd

B, S, D, H = 16, 8192, 512, 512
NEG_INF = -1e18
NCORES = 8
BPC = B // NCORES          # batches per core = 2
NCHUNK = 8                 # chunks per batch
TPC = 8                    # s-tiles (128 positions) per chunk
NT = S // 128              # s-tiles per batch = 64

F32 = mybir.dt.float32
F32R = mybir.dt.float32r


def build_program():
    """Build the per-core SPMD Bass program. Same program on all 8 cores."""
    nc = bacc.Bacc("TRN2", target_bir_lowering=False, debug=False)

    key_d = nc.dram_tensor("key", [BPC, S, D], F32R, kind="ExternalInput")
    # mbias[b, p, t] = -1e18 * mask[b, 128*t + p]  (host precomputed)
    mb_d = nc.dram_tensor("mbias", [BPC, 128, NT], F32, kind="ExternalInput")
    # v replicated across partitions
    vrep_d = nc.dram_tensor("vrep", [128, D], F32, kind="ExternalInput")

    # s_out[b, p, t] = raw score v.key at s = 128*t + p  (unmasked, no qs)
    s_out = nc.dram_tensor("s_out", [BPC, 128, NT], F32, kind="ExternalOutput")
    e_out = nc.dram_tensor("e_out", [BPC, 128, NT], F32R, kind="ExternalOutput")
    ctx_out = nc.dram_tensor("ctx_out", [BPC, 1, D], F32, kind="ExternalOutput")

    # key[b] viewed as [p=128, t=64, d=512]: s = 128*t + p
    key_v = key_d.ap().rearrange("b (t p) d -> b p t d", p=128)

    with tile.TileContext(nc) as tc, ExitStack() as es:
        consts = es.enter_context(tc.tile_pool(name="consts", bufs=1))
        key_pool = es.enter_context(tc.tile_pool(name="key", bufs=10))
        dummy_pool = es.enter_context(tc.tile_pool(name="dummy", bufs=1))
        mb_pool = es.enter_context(tc.tile_pool(name="mb", bufs=2))
        sraw_pool = es.enter_context(tc.tile_pool(name="sraw", bufs=4))
        e_pool = es.enter_context(tc.tile_pool(name="e", bufs=4))
        ctx_sb_pool = es.enter_context(tc.tile_pool(name="ctxsb", bufs=2))
        ctx_ps_pool = es.enter_context(tc.tile_pool(name="ctx_ps", bufs=1, space="PSUM"))

        vrep = consts.tile([128, D], F32)
        nc.scalar.dma_start(out=vrep, in_=vrep_d.ap())
        dummy = dummy_pool.tile([128, 1], F32)

        for b in range(BPC):
            ctx_ps = ctx_ps_pool.tile([1, D], F32)
            mb_b = mb_pool.tile([128, NT], F32)
            nc.scalar.dma_start(out=mb_b, in_=mb_d.ap()[b])

            for c in range(NCHUNK):
                cs = slice(c * TPC, (c + 1) * TPC)
                key_t = key_pool.tile([128, TPC, D], F32R)
                nc.sync.dma_start(
                    out=key_t,
                    in_=key_v[b, :, c * TPC:(c + 1) * TPC, :],
                )
                # fused (key*v) multiply + free-dim reduce: one DVE op/tile
                sraw_c = sraw_pool.tile([128, TPC], F32)
                for tl in range(TPC):
                    nc.vector.scalar_tensor_tensor(
                        out=dummy.broadcast_to(key_t[:, tl, :].shape),
                        in0=key_t[:, tl, :].bitcast(F32),
                        scalar=1.0,
                        in1=vrep,
                        op0=mybir.AluOpType.mult,
                        op1=mybir.AluOpType.mult,
                        accum_out=sraw_c[:, tl:tl + 1],
                    )
                # e = exp(raw + maskbias): mask folds into per-partition bias
                e_c = e_pool.tile([128, TPC], F32R)
                for tl in range(TPC):
                    t = c * TPC + tl
                    nc.scalar.activation(
                        out=e_c[:, tl:tl + 1],
                        in_=sraw_c[:, tl:tl + 1],
                        func=mybir.ActivationFunctionType.Exp,
                        bias=mb_b[:, t:t + 1],
                        scale=1.0,
                    )
                nc.scalar.dma_start(out=s_out.ap()[b, :, cs], in_=sraw_c)
                nc.scalar.dma_start(out=e_out.ap()[b, :, cs], in_=e_c)
                for tl in range(TPC):
                    nc.tensor.matmul(
                        ctx_ps[:, :],
                        lhsT=e_c[:, tl:tl + 1],
                        rhs=key_t[:, tl, :],
                        start=(c == 0 and tl == 0),
                        stop=(c == NCHUNK - 1 and tl == TPC - 1),
                    )

            ctx_sb = ctx_sb_pool.tile([1, D], F32)
            nc.scalar.copy(out=ctx_sb, in_=ctx_ps)
            nc.scalar.dma_start(out=ctx_out.ap()[b, :, :], in_=ctx_sb)

    nc.compile()
    return nc


def host_prep(query, key, mask, Wq, bq, Wk, bk, w_out):
    """Host-side precompute; returns (in_maps, qs)."""
    w64 = w_out.astype(np.float64)
    v = (Wk.astype(np.float64).T @ w64).astype(np.float32)          # (D,)
    qs = (query.astype(np.float64) @ Wq.astype(np.float64).T
          + bq.astype(np.float64) + bk.astype(np.float64)) @ w64     # (B,)

    vrep = np.ascontiguousarray(np.broadcast_to(v, (128, D)))

    # mbias[b, p, t] = -1e18 * mask[b, 128*t + p]
    m = mask.reshape(B, NT, 128).transpose(0, 2, 1)   # (B, 128, NT) bool
    mbias = np.ascontiguousarray(
        np.where(m, np.float32(NEG_INF), np.float32(0.0)).astype(np.float32)
    )
    key_f32 = np.ascontiguousarray(key.astype(np.float32))

    in_maps = []
    for c in range(NCORES):
        lo = c * BPC
        in_maps.append({
            "key": key_f32[lo:lo + BPC],
            "mbias": mbias[lo:lo + BPC],
            "vrep": vrep,
        })
    return in_maps, qs


def host_finish(results, qs, Wk, bk, mask):
    """Gather per-core outputs into (attn, attn_weight)."""
    attn_weight = np.empty((B, S), dtype=np.float32)
    attn = np.empty((B, H), dtype=np.float32)
    Wk64 = Wk.astype(np.float64)
    bk64 = bk.astype(np.float64)
    for c in range(NCORES):
        # device layout [p, t] -> s = 128*t + p
        s_o = results[c]["s_out"].transpose(0, 2, 1).reshape(BPC, S)
        e_o = results[c]["e_out"].transpose(0, 2, 1).reshape(BPC, S)
        ctx = results[c]["ctx_out"].reshape(BPC, D).astype(np.float64)
        for i in range(BPC):
            b = c * BPC + i
            attn_weight[b] = np.where(
                mask[b], np.float32(NEG_INF), s_o[i] + np.float32(qs[b])
            )
            Z = e_o[i].astype(np.float64).sum()
            attn[b] = (Wk64 @ (ctx[i] / Z) + bk64).astype(np.float32)
    return attn, attn_weight


_NC_CACHE = []


def kernel(query, key, mask, Wq, bq, Wk, bk, w_out):
    query = np.asarray(query)
    key = np.asarray(key)
    mask = np.asarray(mask)
    Wq = np.asarray(Wq)
    bq = np.asarray(bq)
    Wk = np.asarray(Wk)
    bk = np.asarray(bk)
    w_out = np.asarray(w_out)

    if not _NC_CACHE:
        _NC_CACHE.append(build_program())
    nc = _NC_CACHE[0]

    in_maps, qs = host_prep(query, key, mask, Wq, bq, Wk, bk, w_out)
    res = run_bass_kernel_spmd(nc, in_maps, list(range(NCORES)))
    return host_finish(res.results, qs, Wk, bk, mask)


# revision 16
# speedup vs baseline: 1.1429x; 1.1429x over previous
"""Trainium2 Bass kernel for FFN (additive) attention.

Reference computation (B=16, S=8192, D=512, H=512):
    q_h = query @ Wq.T + bq                      # (B, H)
    k_h = einsum("bsd,hd->bsh", key, Wk) + bk    # (B, S, H)
    attn_weight = einsum("bsh,h->bs", q_h[:,None,:] + k_h, w_out)
    attn_weight = where(mask, -1e18, attn_weight)
    attn_prob = softmax(attn_weight, -1)
    attn = einsum("bsh,bs->bh", k_h, attn_prob)
    return attn, attn_weight

Key identity: with v = Wk.T @ w_out (D,) and qs[b] = w_out . (Wq@q[b] + bq + bk),
    attn_weight[b,s] = v . key[b,s] + qs[b]
and softmax is invariant to the per-row constant qs[b], while
    attn[b] = Wk @ (sum_s p[b,s] key[b,s]) + bk.
So the device streams `key` (the 256MB input) exactly once:
    pass 1 (DVE): raw[b,s] = v . key[b,s] via one fused
                  scalar_tensor_tensor(+accum) per [128,512] tile on the
                  natural [s-partition, d-free] layout
    e = exp(raw + mask*(-1e18)) on ACT, mask folded into the per-tile
        per-partition activation bias  (no max subtraction: |raw| <~ 45)
    pass 2 (PE):  ctx[b,d] = sum_s e[b,s] key[b,s,d] as fp32r matmuls with
                  e as the 1-column stationary operand
Host applies the mask to the raw scores and finishes with the tiny (D,H)
projections in float64.

DMA queues: key loads own the Sync FIFO (so prefetch never blocks behind
output DMAs); outputs issue from the ACT HWDGE queue.

Sharding: data-parallel over batch, 2 batches per core on 8 cores.
Device layouts use s = 128*t + p (p = partition, t = 0..63 column).
"""

from contextlib import ExitStack

import numpy as np

import concourse.bass as bass
import concourse.bacc as bacc
import concourse.tile as tile
import concourse.mybir as mybir
from concourse.bass_utils import run_bass_kernel_spmd

B, S, D, H = 16, 8192, 512, 512
NEG_INF = -1e18
NCORES = 8
BPC = B // NCORES          # batches per core = 2
NCHUNK = 8                 # chunks per batch
TPC = 8                    # s-tiles (128 positions) per chunk
NT = S // 128              # s-tiles per batch = 64

F32 = mybir.dt.float32
F32R = mybir.dt.float32r


def build_program():
    """Build the per-core SPMD Bass program. Same program on all 8 cores."""
    nc = bacc.Bacc("TRN2", target_bir_lowering=False, debug=False)

    key_d = nc.dram_tensor("key", [BPC, S, D], F32R, kind="ExternalInput")
    # mbias[b, p, t] = -1e18 * mask[b, 128*t + p]  (host precomputed)
    mb_d = nc.dram_tensor("mbias", [BPC, 128, NT], F32, kind="ExternalInput")
    # v replicated across partitions
    vrep_d = nc.dram_tensor("vrep", [128, D], F32, kind="ExternalInput")

    # s_out[b, p, t] = raw score v.key at s = 128*t + p  (unmasked, no qs)
    s_out = nc.dram_tensor("s_out", [BPC, 128, NT], F32, kind="ExternalOutput")
    e_out = nc.dram_tensor("e_out", [BPC, 128, NT], F32R, kind="ExternalOutput")
    ctx_out = nc.dram_tensor("ctx_out", [BPC, 1, D], F32, kind="ExternalOutput")

    # key[b] viewed as [p=128, t=64, d=512]: s = 128*t + p
    key_v = key_d.ap().rearrange("b (t p) d -> b p t d", p=128)

    with tile.TileContext(nc) as tc, ExitStack() as es:
        consts = es.enter_context(tc.tile_pool(name="consts", bufs=1))
        key_pool = es.enter_context(tc.tile_pool(name="key", bufs=8))
        scr_pool = es.enter_context(tc.tile_pool(name="scr", bufs=2))
        mb_pool = es.enter_context(tc.tile_pool(name="mb", bufs=2))
        sraw_pool = es.enter_context(tc.tile_pool(name="sraw", bufs=4))
        e_pool = es.enter_context(tc.tile_pool(name="e", bufs=4))
        ctx_sb_pool = es.enter_context(tc.tile_pool(name="ctxsb", bufs=2))
        ctx_ps_pool = es.enter_context(tc.tile_pool(name="ctx_ps", bufs=1, space="PSUM"))

        vrep = consts.tile([128, D], F32)
        nc.scalar.dma_start(out=vrep, in_=vrep_d.ap())

        for b in range(BPC):
            ctx_ps = ctx_ps_pool.tile([1, D], F32)
            mb_b = mb_pool.tile([128, NT], F32)
            nc.scalar.dma_start(out=mb_b, in_=mb_d.ap()[b])

            for c in range(NCHUNK):
                cs = slice(c * TPC, (c + 1) * TPC)
                key_t = key_pool.tile([128, TPC, D], F32R)
                nc.sync.dma_start(
                    out=key_t,
                    in_=key_v[b, :, c * TPC:(c + 1) * TPC, :],
                )
                # fused (key*v) multiply + free-dim reduce: one DVE op/tile
                scratch = scr_pool.tile([128, TPC, D], F32)
                sraw_c = sraw_pool.tile([128, TPC], F32)
                for tl in range(TPC):
                    nc.vector.scalar_tensor_tensor(
                        out=scratch[:, tl, :],
                        in0=key_t[:, tl, :].bitcast(F32),
                        scalar=1.0,
                        in1=vrep,
                        op0=mybir.AluOpType.mult,
                        op1=mybir.AluOpType.mult,
                        accum_out=sraw_c[:, tl:tl + 1],
                    )
                # e = exp(raw + maskbias): mask folds into per-partition bias
                e_c = e_pool.tile([128, TPC], F32R)
                for tl in range(TPC):
                    t = c * TPC + tl
                    nc.scalar.activation(
                        out=e_c[:, tl:tl + 1],
                        in_=sraw_c[:, tl:tl + 1],
                        func=mybir.ActivationFunctionType.Exp,
                        bias=mb_b[:, t:t + 1],
                        scale=1.0,
                    )
                nc.scalar.dma_start(out=s_out.ap()[b, :, cs], in_=sraw_c)
                nc.scalar.dma_start(out=e_out.ap()[b, :, cs], in_=e_c)
                for tl in range(TPC):
                    nc.tensor.matmul(
                        ctx_ps[:, :],
                        lhsT=e_c[:, tl:tl + 1],
                        rhs=key_t[:, tl, :],
                        start=(c == 0 and tl == 0),
                        stop=(c == NCHUNK - 1 and tl == TPC - 1),
                    )

            ctx_sb = ctx_sb_pool.tile([1, D], F32)
            nc.scalar.copy(out=ctx_sb, in_=ctx_ps)
            nc.scalar.dma_start(out=ctx_out.ap()[b, :, :], in_=ctx_sb)

    nc.compile()
    return nc


def host_prep(query, key, mask, Wq, bq, Wk, bk, w_out):
    """Host-side precompute; returns (in_maps, qs)."""
    w64 = w_out.astype(np.float64)
    v = (Wk.astype(np.float64).T @ w64).astype(np.float32)          # (D,)
    qs = (query.astype(np.float64) @ Wq.astype(np.float64).T
          + bq.astype(np.float64) + bk.astype(np.float64)) @ w64     # (B,)

    vrep = np.ascontiguousarray(np.broadcast_to(v, (128, D)))

    # mbias[b, p, t] = -1e18 * mask[b, 128*t + p]
    m = mask.reshape(B, NT, 128).transpose(0, 2, 1)   # (B, 128, NT) bool
    mbias = np.ascontiguousarray(
        np.where(m, np.float32(NEG_INF), np.float32(0.0)).astype(np.float32)
    )
    key_f32 = np.ascontiguousarray(key.astype(np.float32))

    in_maps = []
    for c in range(NCORES):
        lo = c * BPC
        in_maps.append({
            "key": key_f32[lo:lo + BPC],
            "mbias": mbias[lo:lo + BPC],
            "vrep": vrep,
        })
    return in_maps, qs


def host_finish(results, qs, Wk, bk, mask):
    """Gather per-core outputs into (attn, attn_weight)."""
    attn_weight = np.empty((B, S), dtype=np.float32)
    attn = np.empty((B, H), dtype=np.float32)
    Wk64 = Wk.astype(np.float64)
    bk64 = bk.astype(np.float64)
    for c in range(NCORES):
        # device layout [p, t] -> s = 128*t + p
        s_o = results[c]["s_out"].transpose(0, 2, 1).reshape(BPC, S)
        e_o = results[c]["e_out"].transpose(0, 2, 1).reshape(BPC, S)
        ctx = results[c]["ctx_out"].reshape(BPC, D).astype(np.float64)
        for i in range(BPC):
            b = c * BPC + i
            attn_weight[b] = np.where(
                mask[b], np.float32(NEG_INF), s_o[i] + np.float32(qs[b])
            )
            Z = e_o[i].astype(np.float64).sum()
            attn[b] = (Wk64 @ (ctx[i] / Z) + bk64).astype(np.float32)
    return attn, attn_weight


_NC_CACHE = []


def kernel(query, key, mask, Wq, bq, Wk, bk, w_out):
    query = np.asarray(query)
    key = np.asarray(key)
    mask = np.asarray(mask)
    Wq = np.asarray(Wq)
    bq = np.asarray(bq)
    Wk = np.asarray(Wk)
    bk = np.asarray(bk)
    w_out = np.asarray(w_out)

    if not _NC_CACHE:
        _NC_CACHE.append(build_program())
    nc = _NC_CACHE[0]

    in_maps, qs = host_prep(query, key, mask, Wq, bq, Wk, bk, w_out)
    res = run_bass_kernel_spmd(nc, in_maps, list(range(NCORES)))
    return host_finish(res.results, qs, Wk, bk, mask)


# revision 17
# speedup vs baseline: 1.1730x; 1.0263x over previous
"""Trainium2 Bass kernel for FFN (additive) attention.

Reference computation (B=16, S=8192, D=512, H=512):
    q_h = query @ Wq.T + bq                      # (B, H)
    k_h = einsum("bsd,hd->bsh", key, Wk) + bk    # (B, S, H)
    attn_weight = einsum("bsh,h->bs", q_h[:,None,:] + k_h, w_out)
    attn_weight = where(mask, -1e18, attn_weight)
    attn_prob = softmax(attn_weight, -1)
    attn = einsum("bsh,bs->bh", k_h, attn_prob)
    return attn, attn_weight

Key identity: with v = Wk.T @ w_out (D,) and qs[b] = w_out . (Wq@q[b] + bq + bk),
    attn_weight[b,s] = v . key[b,s] + qs[b]
and softmax is invariant to the per-row constant qs[b], while
    attn[b] = Wk @ (sum_s p[b,s] key[b,s]) + bk.
So the device streams `key` (the 256MB input) exactly once:
    pass 1 (DVE): raw[b,s] = v . key[b,s] via one fused
                  scalar_tensor_tensor(+accum) per [128,512] tile on the
                  natural [s-partition, d-free] layout
    e = exp(raw + mask*(-1e18)) on ACT, mask folded into the per-tile
        per-partition activation bias  (no max subtraction: |raw| <~ 45)
    pass 2 (PE):  ctx[b,d] = sum_s e[b,s] key[b,s,d] as fp32r matmuls with
                  e as the 1-column stationary operand
Host applies the mask to the raw scores and finishes with the tiny (D,H)
projections in float64.

DMA queues: key loads own the Sync FIFO (so prefetch never blocks behind
output DMAs); outputs issue from the ACT HWDGE queue.

Sharding: data-parallel over batch, 2 batches per core on 8 cores.
Device layouts use s = 128*t + p (p = partition, t = 0..63 column).
"""

from contextlib import ExitStack

import numpy as np

import concourse.bass as bass
import concourse.bacc as bacc
import concourse.tile as tile
import concourse.mybir as mybir
from concourse.bass_utils import run_bass_kernel_spmd

B, S, D, H = 16, 8192, 512, 512
NEG_INF = -1e18
NCORES = 8
BPC = B // NCORES          # batches per core = 2
NCHUNK = 8                 # chunks per batch
TPC = 8                    # s-tiles (128 positions) per chunk
NT = S // 128              # s-tiles per batch = 64

F32 = mybir.dt.float32
F32R = mybir.dt.float32r


def build_program():
    """Build the per-core SPMD Bass program. Same program on all 8 cores."""
    nc = bacc.Bacc("TRN2", target_bir_lowering=False, debug=False)

    key_d = nc.dram_tensor("key", [BPC, S, D], F32R, kind="ExternalInput")
    # mbias[b, p, t] = -1e18 * mask[b, 128*t + p]  (host precomputed)
    mb_d = nc.dram_tensor("mbias", [BPC, 128, NT], F32, kind="ExternalInput")
    # v replicated across partitions
    vrep_d = nc.dram_tensor("vrep", [128, D], F32, kind="ExternalInput")

    # s_out[b, p, t] = raw score v.key at s = 128*t + p  (unmasked, no qs)
    s_out = nc.dram_tensor("s_out", [BPC, 128, NT], F32, kind="ExternalOutput")
    e_out = nc.dram_tensor("e_out", [BPC, 128, NT], F32R, kind="ExternalOutput")
    ctx_out = nc.dram_tensor("ctx_out", [BPC, 1, D], F32, kind="ExternalOutput")

    # key[b] viewed as [p=128, t=64, d=512]: s = 128*t + p
    key_v = key_d.ap().rearrange("b (t p) d -> b p t d", p=128)

    with tile.TileContext(nc) as tc, ExitStack() as es:
        consts = es.enter_context(tc.tile_pool(name="consts", bufs=1))
        key_pool = es.enter_context(tc.tile_pool(name="key", bufs=8))
        scr_pool = es.enter_context(tc.tile_pool(name="scr", bufs=2))
        mb_pool = es.enter_context(tc.tile_pool(name="mb", bufs=2))
        sraw_pool = es.enter_context(tc.tile_pool(name="sraw", bufs=4))
        e_pool = es.enter_context(tc.tile_pool(name="e", bufs=4))
        ctx_sb_pool = es.enter_context(tc.tile_pool(name="ctxsb", bufs=2))
        ctx_ps_pool = es.enter_context(tc.tile_pool(name="ctx_ps", bufs=1, space="PSUM"))

        vrep = consts.tile([128, D], F32)
        nc.scalar.dma_start(out=vrep, in_=vrep_d.ap())

        for b in range(BPC):
            ctx_ps = ctx_ps_pool.tile([1, D], F32)
            mb_b = mb_pool.tile([128, NT], F32)
            nc.scalar.dma_start(out=mb_b, in_=mb_d.ap()[b])

            for c in range(NCHUNK):
                cs = slice(c * TPC, (c + 1) * TPC)
                key_t = key_pool.tile([128, TPC, D], F32R)
                h = TPC // 2
                nc.sync.dma_start(
                    out=key_t[:, 0:h, :],
                    in_=key_v[b, :, c * TPC:c * TPC + h, :],
                )
                nc.sync.dma_start(
                    out=key_t[:, h:TPC, :],
                    in_=key_v[b, :, c * TPC + h:(c + 1) * TPC, :],
                )
                # fused (key*v) multiply + free-dim reduce: one DVE op/tile
                scratch = scr_pool.tile([128, TPC, D], F32)
                sraw_c = sraw_pool.tile([128, TPC], F32)
                for tl in range(TPC):
                    nc.vector.scalar_tensor_tensor(
                        out=scratch[:, tl, :],
                        in0=key_t[:, tl, :].bitcast(F32),
                        scalar=1.0,
                        in1=vrep,
                        op0=mybir.AluOpType.mult,
                        op1=mybir.AluOpType.mult,
                        accum_out=sraw_c[:, tl:tl + 1],
                    )
                # e = exp(raw + maskbias): mask folds into per-partition bias
                e_c = e_pool.tile([128, TPC], F32R)
                for tl in range(TPC):
                    t = c * TPC + tl
                    nc.scalar.activation(
                        out=e_c[:, tl:tl + 1],
                        in_=sraw_c[:, tl:tl + 1],
                        func=mybir.ActivationFunctionType.Exp,
                        bias=mb_b[:, t:t + 1],
                        scale=1.0,
                    )
                nc.scalar.dma_start(out=s_out.ap()[b, :, cs], in_=sraw_c)
                nc.scalar.dma_start(out=e_out.ap()[b, :, cs], in_=e_c)
                for tl in range(TPC):
                    nc.tensor.matmul(
                        ctx_ps[:, :],
                        lhsT=e_c[:, tl:tl + 1],
                        rhs=key_t[:, tl, :],
                        start=(c == 0 and tl == 0),
                        stop=(c == NCHUNK - 1 and tl == TPC - 1),
                    )

            ctx_sb = ctx_sb_pool.tile([1, D], F32)
            nc.scalar.copy(out=ctx_sb, in_=ctx_ps)
            nc.scalar.dma_start(out=ctx_out.ap()[b, :, :], in_=ctx_sb)

    nc.compile()
    return nc


def host_prep(query, key, mask, Wq, bq, Wk, bk, w_out):
    """Host-side precompute; returns (in_maps, qs)."""
    w64 = w_out.astype(np.float64)
    v = (Wk.astype(np.float64).T @ w64).astype(np.float32)          # (D,)
    qs = (query.astype(np.float64) @ Wq.astype(np.float64).T
          + bq.astype(np.float64) + bk.astype(np.float64)) @ w64     # (B,)

    vrep = np.ascontiguousarray(np.broadcast_to(v, (128, D)))

    # mbias[b, p, t] = -1e18 * mask[b, 128*t + p]
    m = mask.reshape(B, NT, 128).transpose(0, 2, 1)   # (B, 128, NT) bool
    mbias = np.ascontiguousarray(
        np.where(m, np.float32(NEG_INF), np.float32(0.0)).astype(np.float32)
    )
    key_f32 = np.ascontiguousarray(key.astype(np.float32))

    in_maps = []
    for c in range(NCORES):
        lo = c * BPC
        in_maps.append({
            "key": key_f32[lo:lo + BPC],
            "mbias": mbias[lo:lo + BPC],
            "vrep": vrep,
        })
    return in_maps, qs


def host_finish(results, qs, Wk, bk, mask):
    """Gather per-core outputs into (attn, attn_weight)."""
    attn_weight = np.empty((B, S), dtype=np.float32)
    attn = np.empty((B, H), dtype=np.float32)
    Wk64 = Wk.astype(np.float64)
    bk64 = bk.astype(np.float64)
    for c in range(NCORES):
        # device layout [p, t] -> s = 128*t + p
        s_o = results[c]["s_out"].transpose(0, 2, 1).reshape(BPC, S)
        e_o = results[c]["e_out"].transpose(0, 2, 1).reshape(BPC, S)
        ctx = results[c]["ctx_out"].reshape(BPC, D).astype(np.float64)
        for i in range(BPC):
            b = c * BPC + i
            attn_weight[b] = np.where(
                mask[b], np.float32(NEG_INF), s_o[i] + np.float32(qs[b])
            )
            Z = e_o[i].astype(np.float64).sum()
            attn[b] = (Wk64 @ (ctx[i] / Z) + bk64).astype(np.float32)
    return attn, attn_weight


_NC_CACHE = []


def kernel(query, key, mask, Wq, bq, Wk, bk, w_out):
    query = np.asarray(query)
    key = np.asarray(key)
    mask = np.asarray(mask)
    Wq = np.asarray(Wq)
    bq = np.asarray(bq)
    Wk = np.asarray(Wk)
    bk = np.asarray(bk)
    w_out = np.asarray(w_out)

    if not _NC_CACHE:
        _NC_CACHE.append(build_program())
    nc = _NC_CACHE[0]

    in_maps, qs = host_prep(query, key, mask, Wq, bq, Wk, bk, w_out)
    res = run_bass_kernel_spmd(nc, in_maps, list(range(NCORES)))
    return host_finish(res.results, qs, Wk, bk, mask)


# revision 18
# speedup vs baseline: 1.1774x; 1.0038x over previous
"""Trainium2 Bass kernel for FFN (additive) attention.

Reference computation (B=16, S=8192, D=512, H=512):
    q_h = query @ Wq.T + bq                      # (B, H)
    k_h = einsum("bsd,hd->bsh", key, Wk) + bk    # (B, S, H)
    attn_weight = einsum("bsh,h->bs", q_h[:,None,:] + k_h, w_out)
    attn_weight = where(mask, -1e18, attn_weight)
    attn_prob = softmax(attn_weight, -1)
    attn = einsum("bsh,bs->bh", k_h, attn_prob)
    return attn, attn_weight

Key identity: with v = Wk.T @ w_out (D,) and qs[b] = w_out . (Wq@q[b] + bq + bk),
    attn_weight[b,s] = v . key[b,s] + qs[b]
and softmax is invariant to the per-row constant qs[b], while
    attn[b] = Wk @ (sum_s p[b,s] key[b,s]) + bk.
So the device streams `key` (the 256MB input) exactly once:
    pass 1 (DVE): raw[b,s] = v . key[b,s] via one fused
                  scalar_tensor_tensor(+accum) per [128,512] tile on the
                  natural [s-partition, d-free] layout
    e = exp(raw + mask*(-1e18)) on ACT, mask folded into the per-tile
        per-partition activation bias  (no max subtraction: |raw| <~ 45)
    pass 2 (PE):  ctx[b,d] = sum_s e[b,s] key[b,s,d] as fp32r matmuls with
                  e as the 1-column stationary operand
Host applies the mask to the raw scores and finishes with the tiny (D,H)
projections in float64.

DMA queues: key loads own the Sync FIFO (so prefetch never blocks behind
output DMAs); outputs issue from the ACT HWDGE queue.

Sharding: data-parallel over batch, 2 batches per core on 8 cores.
Device layouts use s = 128*t + p (p = partition, t = 0..63 column).
"""

from contextlib import ExitStack

import numpy as np

import concourse.bacc as bacc
import concourse.tile as tile
import concourse.mybir as mybir
from concourse.bass_utils import run_bass_kernel_spmd

B, S, D, H = 16, 8192, 512, 512
NEG_INF = -1e18
NCORES = 8
BPC = B // NCORES          # batches per core = 2
NCHUNK = 8                 # chunks per batch
TPC = 8                    # s-tiles (128 positions) per chunk
NT = S // 128              # s-tiles per batch = 64

F32 = mybir.dt.float32
F32R = mybir.dt.float32r


def build_program():
    """Build the per-core SPMD Bass program. Same program on all 8 cores."""
    nc = bacc.Bacc("TRN2", target_bir_lowering=False, debug=False)

    key_d = nc.dram_tensor("key", [BPC, S, D], F32R, kind="ExternalInput")
    # mbias[b, p, t] = -1e18 * mask[b, 128*t + p]  (host precomputed)
    mb_d = nc.dram_tensor("mbias", [BPC, 128, NT], F32, kind="ExternalInput")
    # v replicated across partitions
    vrep_d = nc.dram_tensor("vrep", [128, D], F32, kind="ExternalInput")

    # s_out[b, p, t] = raw score v.key at s = 128*t + p  (unmasked, no qs)
    s_out = nc.dram_tensor("s_out", [BPC, 128, NT], F32, kind="ExternalOutput")
    e_out = nc.dram_tensor("e_out", [BPC, 128, NT], F32R, kind="ExternalOutput")
    ctx_out = nc.dram_tensor("ctx_out", [BPC, 1, D], F32, kind="ExternalOutput")

    # key[b] viewed as [p=128, t=64, d=512]: s = 128*t + p
    key_v = key_d.ap().rearrange("b (t p) d -> b p t d", p=128)

    with tile.TileContext(nc) as tc, ExitStack() as es:
        consts = es.enter_context(tc.tile_pool(name="consts", bufs=1))
        key_pool = es.enter_context(tc.tile_pool(name="key", bufs=8))
        scr_pool = es.enter_context(tc.tile_pool(name="scr", bufs=2))
        mb_pool = es.enter_context(tc.tile_pool(name="mb", bufs=2))
        sraw_pool = es.enter_context(tc.tile_pool(name="sraw", bufs=4))
        e_pool = es.enter_context(tc.tile_pool(name="e", bufs=4))
        ctx_sb_pool = es.enter_context(tc.tile_pool(name="ctxsb", bufs=2))
        ctx_ps_pool = es.enter_context(tc.tile_pool(name="ctx_ps", bufs=1, space="PSUM"))

        vrep = consts.tile([128, D], F32)
        nc.scalar.dma_start(out=vrep, in_=vrep_d.ap())

        for b in range(BPC):
            ctx_ps = ctx_ps_pool.tile([1, D], F32)
            mb_b = mb_pool.tile([128, NT], F32)
            nc.scalar.dma_start(out=mb_b, in_=mb_d.ap()[b])

            for c in range(NCHUNK):
                cs = slice(c * TPC, (c + 1) * TPC)
                key_t = key_pool.tile([128, TPC, D], F32R)
                h = TPC // 2
                nc.sync.dma_start(
                    out=key_t[:, 0:h, :],
                    in_=key_v[b, :, c * TPC:c * TPC + h, :],
                )
                nc.sync.dma_start(
                    out=key_t[:, h:TPC, :],
                    in_=key_v[b, :, c * TPC + h:(c + 1) * TPC, :],
                )
                # fused (key*v) multiply + free-dim reduce: one DVE op/tile
                scratch = scr_pool.tile([128, TPC, D], F32)
                sraw_c = sraw_pool.tile([128, TPC], F32)
                for tl in range(TPC):
                    nc.vector.scalar_tensor_tensor(
                        out=scratch[:, tl, :],
                        in0=key_t[:, tl, :].bitcast(F32),
                        scalar=1.0,
                        in1=vrep,
                        op0=mybir.AluOpType.mult,
                        op1=mybir.AluOpType.mult,
                        accum_out=sraw_c[:, tl:tl + 1],
                    )
                # e = exp(raw + maskbias): mask folds into per-partition bias
                e_c = e_pool.tile([128, TPC], F32R)
                for tl in range(TPC):
                    t = c * TPC + tl
                    nc.scalar.activation(
                        out=e_c[:, tl:tl + 1],
                        in_=sraw_c[:, tl:tl + 1],
                        func=mybir.ActivationFunctionType.Exp,
                        bias=mb_b[:, t:t + 1],
                        scale=1.0,
                    )
                nc.scalar.dma_start(out=s_out.ap()[b, :, cs], in_=sraw_c)
                nc.scalar.dma_start(out=e_out.ap()[b, :, cs], in_=e_c)
                for tl in range(TPC):
                    nc.tensor.matmul(
                        ctx_ps[:, :],
                        lhsT=e_c[:, tl:tl + 1],
                        rhs=key_t[:, tl, :],
                        start=(c == 0 and tl == 0),
                        stop=(c == NCHUNK - 1 and tl == TPC - 1),
                    )

            ctx_sb = ctx_sb_pool.tile([1, D], F32)
            nc.scalar.copy(out=ctx_sb, in_=ctx_ps)
            nc.scalar.dma_start(out=ctx_out.ap()[b, :, :], in_=ctx_sb)

    nc.compile()
    return nc


def host_prep(query, key, mask, Wq, bq, Wk, bk, w_out):
    """Host-side precompute; returns (in_maps, qs)."""
    w64 = w_out.astype(np.float64)
    v = (Wk.astype(np.float64).T @ w64).astype(np.float32)          # (D,)
    qs = (query.astype(np.float64) @ Wq.astype(np.float64).T
          + bq.astype(np.float64) + bk.astype(np.float64)) @ w64     # (B,)

    vrep = np.ascontiguousarray(np.broadcast_to(v, (128, D)))

    # mbias[b, p, t] = -1e18 * mask[b, 128*t + p]
    m = mask.reshape(B, NT, 128).transpose(0, 2, 1)   # (B, 128, NT) bool
    mbias = np.ascontiguousarray(
        np.where(m, np.float32(NEG_INF), np.float32(0.0)).astype(np.float32)
    )
    key_f32 = np.ascontiguousarray(key.astype(np.float32))

    in_maps = []
    for c in range(NCORES):
        lo = c * BPC
        in_maps.append({
            "key": key_f32[lo:lo + BPC],
            "mbias": mbias[lo:lo + BPC],
            "vrep": vrep,
        })
    return in_maps, qs


def host_finish(results, qs, Wk, bk, mask):
    """Gather per-core outputs into (attn, attn_weight)."""
    attn_weight = np.empty((B, S), dtype=np.float32)
    attn = np.empty((B, H), dtype=np.float32)
    Wk64 = Wk.astype(np.float64)
    bk64 = bk.astype(np.float64)
    for c in range(NCORES):
        # device layout [p, t] -> s = 128*t + p
        s_o = results[c]["s_out"].transpose(0, 2, 1).reshape(BPC, S)
        e_o = results[c]["e_out"].transpose(0, 2, 1).reshape(BPC, S)
        ctx = results[c]["ctx_out"].reshape(BPC, D).astype(np.float64)
        for i in range(BPC):
            b = c * BPC + i
            attn_weight[b] = np.where(
                mask[b], np.float32(NEG_INF), s_o[i] + np.float32(qs[b])
            )
            Z = e_o[i].astype(np.float64).sum()
            attn[b] = (Wk64 @ (ctx[i] / Z) + bk64).astype(np.float32)
    return attn, attn_weight


_NC_CACHE = []


def kernel(query, key, mask, Wq, bq, Wk, bk, w_out):
    query = np.asarray(query)
    key = np.asarray(key)
    mask = np.asarray(mask)
    Wq = np.asarray(Wq)
    bq = np.asarray(bq)
    Wk = np.asarray(Wk)
    bk = np.asarray(bk)
    w_out = np.asarray(w_out)

    if not _NC_CACHE:
        _NC_CACHE.append(build_program())
    nc = _NC_CACHE[0]

    in_maps, qs = host_prep(query, key, mask, Wq, bq, Wk, bk, w_out)
    res = run_bass_kernel_spmd(nc, in_maps, list(range(NCORES)))
    return host_finish(res.results, qs, Wk, bk, mask)
